# revision 4
# baseline (speedup 1.0000x reference)
"""BottleneckLSTMCell fused kernel for 8 Trainium2 NeuronCores.

Sharding: data-parallel over batch (B=8 -> 1 image per core). Each core runs
the full cell for its image:

  phase A: xw = dw3x3(x) (+bias folded into the Wy bias); i = Wy @ [h; xw] + b
  phase B: b = dw3x3(i); four 1x1 gate matmuls; LSTM pointwise -> (ch, cc)

All matmul operands are bf16 (PSUM accumulation stays fp32). Depthwise convs
run as 32x32 tile-position matmuls: per 128-ch chunk, 8 PE tiles work
concurrently (diag tiles on the natural image, off-diag tiles on a
partition-rolled copy), each accumulating 9 taps for its (channel-block,
pixel-half) region. Same-column tiles land in different PSUM banks (hardware
requirement). Phases A and B are software-pipelined (B lags A by 2 slabs).
"""

import sys

if '/opt/trn_rl_repo' not in sys.path:
    sys.path.insert(0, '/opt/trn_rl_repo')

import numpy as np
import ml_dtypes

import concourse.bass as bass  # noqa: F401
from concourse import bacc
import concourse.mybir as mybir
from concourse.tile import TileContext
from concourse.bass_utils import run_bass_kernel_spmd

F32 = mybir.dt.float32
BF16 = mybir.dt.bfloat16
NPBF16 = ml_dtypes.bfloat16
AF = mybir.ActivationFunctionType
ALU = mybir.AluOpType

B, CIN, CH, HW = 8, 320, 512, 64
PIX = HW * HW          # 4096
NCORES = 8
NCHUNK = 8             # spatial slabs of 8 rows (512 px)

TAPS = [(t // 3 - 1, t % 3 - 1) for t in range(9)]


def build_nc():
    nc = bacc.Bacc(None, target_bir_lowering=False, num_devices=NCORES)

    xd = nc.dram_tensor("x", (CIN, 66, 66), BF16, kind="ExternalInput")
    hd = nc.dram_tensor("h", (CH, PIX), BF16, kind="ExternalInput")
    cd = nc.dram_tensor("c", (CH, PIX), BF16, kind="ExternalInput")
    wyd = nc.dram_tensor("wy", (128, 7, 512), BF16, kind="ExternalInput")
    wybd = nc.dram_tensor("wyb", (128, 4), F32, kind="ExternalInput")
    wgd = nc.dram_tensor("wg", (128, 16, 512), BF16, kind="ExternalInput")
    dwxd = nc.dram_tensor("dwx", (128, 2, 1152), BF16, kind="ExternalInput")
    dwx2d = nc.dram_tensor("dwx2", (128, 288), BF16, kind="ExternalInput")
    dwid = nc.dram_tensor("dwi", (128, 4, 1152), BF16, kind="ExternalInput")
    ccd = nc.dram_tensor("occ", (CH, PIX), BF16, kind="ExternalOutput")
    chd = nc.dram_tensor("och", (CH, PIX), BF16, kind="ExternalOutput")

    x_ap, h_ap, c_ap = xd.ap(), hd.ap(), cd.ap()
    cc_ap, ch_ap = ccd.ap(), chd.ap()

    with TileContext(nc) as tc:
        with (
            tc.tile_pool(name="persist", bufs=1) as pp,
            tc.tile_pool(name="wB", bufs=1) as wB,
            tc.tile_pool(name="sA", bufs=2) as sA,
            tc.tile_pool(name="sB", bufs=2) as sB,
            tc.tile_pool(name="psdw", bufs=4, space="PSUM") as psdw,
            tc.tile_pool(name="psi", bufs=2, space="PSUM") as psi,
            tc.tile_pool(name="psg", bufs=2, space="PSUM") as psg,
        ):
            # ---- persistent tiles / weights ----
            i_pad = [pp.tile([128, 66, 66], BF16, tag=f"ipad{m}",
                             name=f"ipad{m}") for m in range(4)]
            for m in range(4):
                nc.vector.memset(i_pad[m][:, 0, :], 0.0)
                nc.vector.memset(i_pad[m][:, 65, :], 0.0)
                nc.vector.memset(i_pad[m][:, :, 0], 0.0)
                nc.vector.memset(i_pad[m][:, :, 65], 0.0)

            dwx_t = pp.tile([128, 2, 1152], BF16, tag="dwx", name="dwx")
            for ci in range(2):
                nc.sync.dma_start(out=dwx_t[:, ci, :], in_=dwxd.ap()[:, ci, :])
            dwx2_t = pp.tile([128, 288], BF16, tag="dwx2", name="dwx2")
            nc.sync.dma_start(out=dwx2_t[:], in_=dwx2d.ap())
            wy_t = pp.tile([128, 7, 512], BF16, tag="wy", name="wy")
            for k in range(7):
                nc.sync.dma_start(out=wy_t[:, k, :], in_=wyd.ap()[:, k, :])
            wyb_t = pp.tile([128, 4], F32, tag="wyb", name="wyb")
            nc.sync.dma_start(out=wyb_t[:], in_=wybd.ap())

            def load_wB():
                wg_t = wB.tile([128, 16, 512], BF16, tag="wg", name="wg")
                for k in range(16):
                    nc.sync.dma_start(out=wg_t[:, k, :], in_=wgd.ap()[:, k, :])
                dwi_t = wB.tile([128, 4, 1152], BF16, tag="dwi", name="dwi")
                for ci in range(4):
                    nc.sync.dma_start(out=dwi_t[:, ci, :],
                                      in_=dwid.ap()[:, ci, :])
                return wg_t, dwi_t

            # ---- per-slab input loading (phase A) ----
            def emit_slab_inputs(n):
                r0 = 8 * n
                ht = sA.tile([128, 4, 512], BF16, tag="h", name="h")
                nc.sync.dma_start(
                    out=ht[:],
                    in_=h_ap[:, 512 * n:512 * (n + 1)].rearrange(
                        "(k p) x -> p k x", p=128),
                )
                xp = []
                for ci in range(2):
                    t0 = sA.tile([128, 10, 66], BF16, tag=f"xp{ci}",
                                 name=f"xp{ci}")
                    nc.sync.dma_start(
                        out=t0[:],
                        in_=x_ap[128 * ci:128 * (ci + 1), r0:r0 + 10, :])
                    t1 = sA.tile([128, 10, 66], BF16, tag=f"xq{ci}",
                                 name=f"xq{ci}")
                    # partition-rolled copy: dst p <- src ch (p+32)%128
                    nc.sync.dma_start(
                        out=t1[:96, :, :],
                        in_=x_ap[128 * ci + 32:128 * (ci + 1), r0:r0 + 10, :])
                    nc.sync.dma_start(
                        out=t1[96:128, :, :],
                        in_=x_ap[128 * ci:128 * ci + 32, r0:r0 + 10, :])
                    xp.append((t0, t1))
                # chunk2: 64 ch; natural at p0-63, extra copy at p64-127
                t2 = sA.tile([128, 10, 66], BF16, tag="xp2", name="xp2")
                nc.sync.dma_start(out=t2[0:64, :, :],
                                  in_=x_ap[256:320, r0:r0 + 10, :])
                nc.sync.dma_start(out=t2[64:128, :, :],
                                  in_=x_ap[256:320, r0:r0 + 10, :])
                xp.append(t2)
                return ht, xp

            # ---- tiled depthwise: one 128-ch chunk -> out_sb [128,8,64] ----
            def dw_chunk_full(wt_ap_fn, img_nat, img_rot, row_off, out_sb,
                              evac_eng):
                """wt_ap_fn(I, t) -> lhsT AP [32, 32] for (tile-row I, tap t,
                col J); img_nat/img_rot: [128, >=10, 66] window tiles
                (rot = rolled by 32 partitions); row_off: first image row of
                this slab inside the tile."""
                pss = []
                for r in range(2):
                    ps = psdw.tile([128, 8, 64], F32, tag="psdw", name="psdw")
                    pss.append(ps)
                for t, (dy, dx) in enumerate(TAPS):
                    for r in range(2):
                        img = img_nat if r == 0 else img_rot
                        for J in range(4):
                            I = (J - r) % 4
                            rr = row_off + 4 * r + dy
                            nc.tensor.matmul(
                                pss[r][32 * J:32 * (J + 1), 0:4, :],
                                wt_ap_fn(I, t, J),
                                img[32 * I:32 * (I + 1), rr:rr + 4,
                                    1 + dx:65 + dx],
                                start=(t == 0), stop=(t == 8),
                                tile_position=(32 * I, 32 * J),
                            )
                for r in range(2):
                    evac_eng(out_sb[:, 4 * r:4 * r + 4, :], pss[r][:, 0:4, :])

            def dw_chunk2(img2, row_off, out_sb, evac_eng):
                """64-ch chunk: J in {0,1}; copies at I = J + 2r."""
                pss = []
                for r in range(2):
                    ps = psdw.tile([128, 8, 64], F32, tag="psdw", name="psdw")
                    pss.append(ps)
                for t, (dy, dx) in enumerate(TAPS):
                    for r in range(2):
                        for J in range(2):
                            I = J + 2 * r
                            rr = row_off + 4 * r + dy
                            nc.tensor.matmul(
                                pss[r][32 * J:32 * (J + 1), 0:4, :],
                                dwx2_t[32 * I:32 * (I + 1),
                                       32 * t:32 * (t + 1)],
                                img2[32 * I:32 * (I + 1), rr:rr + 4,
                                     1 + dx:65 + dx],
                                start=(t == 0), stop=(t == 8),
                                tile_position=(32 * I, 32 * J),
                            )
                for r in range(2):
                    evac_eng(out_sb[0:64, 4 * r:4 * r + 4, :],
                             pss[r][0:64, 0:4, :])

            def act_evac(dst, src):
                nc.scalar.copy(dst, src)

            def vec_evac(dst, src):
                nc.vector.tensor_copy(out=dst, in0=src)

            # ---- phase A for slab n ----
            def emit_A(n, ht, xp):
                r0 = 8 * n
                xw_all = sA.tile([128, 3, 8, 64], BF16, tag="xw", name="xw")
                for ci in range(2):
                    nat, rot = xp[ci]
                    dw_chunk_full(
                        lambda I, t, J, _c=ci: dwx_t[
                            32 * I:32 * (I + 1), _c,
                            128 * t + 32 * J:128 * t + 32 * (J + 1)],
                        nat, rot, 1, xw_all[:, ci, :, :], act_evac)
                dw_chunk2(xp[2], 1, xw_all[:, 2, :, :], act_evac)

                for m in range(4):
                    ps = psi.tile([128, 512], F32, tag="psi", name="psi")
                    for k in range(4):  # h chunks first (ready earlier)
                        nc.tensor.matmul(
                            ps[:, :], wy_t[:, k, 128 * m:128 * (m + 1)],
                            ht[:, k, :], start=(k == 0), stop=False)
                    for j in range(3):
                        pc = 128 if j < 2 else 64
                        nc.tensor.matmul(
                            ps[:, :],
                            wy_t[:pc, 4 + j, 128 * m:128 * (m + 1)],
                            xw_all[:pc, j, :, :],
                            start=False, stop=(j == 2))
                    nc.scalar.activation(
                        i_pad[m][:, 1 + r0:9 + r0, 1:65], ps[:, :],
                        AF.Identity, bias=wyb_t[:, m:m + 1], scale=1.0)

            # ---- rolled i-window copies for slab n (phase B inputs) ----
            def emit_C(n):
                r0 = 8 * n
                iws = []
                for ci in range(4):
                    iw = sB.tile([128, 10, 66], BF16, tag=f"iw{ci}",
                                 name=f"iw{ci}")
                    nc.gpsimd.dma_start(
                        out=iw[:96, :, :],
                        in_=i_pad[ci][32:128, r0:r0 + 10, :])
                    nc.gpsimd.dma_start(
                        out=iw[96:128, :, :],
                        in_=i_pad[ci][0:32, r0:r0 + 10, :])
                    iws.append(iw)
                return iws

            # ---- phase B for slab n ----
            def emit_B(n, iws, wg_t, dwi_t):
                r0 = 8 * n
                b_sb = []
                for ci in range(4):
                    bt = sB.tile([128, 8, 64], BF16, tag=f"b{ci}",
                                 name=f"b{ci}")
                    dw_chunk_full(
                        lambda I, t, J, _c=ci: dwi_t[
                            32 * I:32 * (I + 1), _c,
                            128 * t + 32 * J:128 * t + 32 * (J + 1)],
                        i_pad[ci][:, r0:r0 + 10, :], iws[ci],
                        1, bt, act_evac)
                    b_sb.append(bt)

                for m in range(4):
                    c_t = sB.tile([128, 512], BF16, tag="c", name="c")
                    nc.sync.dma_start(
                        out=c_t[:],
                        in_=c_ap[128 * m:128 * (m + 1),
                                 512 * n:512 * (n + 1)])
                    sig = []
                    for g in range(4):  # 0=i 1=f 2=c 3=o
                        ps = psg.tile([128, 512], F32, tag="psg", name="psg")
                        for k in range(4):
                            nc.tensor.matmul(
                                ps[:, :],
                                wg_t[:, 4 * g + k, 128 * m:128 * (m + 1)],
                                b_sb[k][:, :, :],
                                start=(k == 0), stop=(k == 3))
                        st = sB.tile([128, 512], BF16, tag=f"sg{g}",
                                     name=f"sg{g}")
                        if g == 2:
                            nc.vector.tensor_scalar(
                                out=st[:, :], in0=ps[:, :],
                                scalar1=0.0, scalar2=6.0,
                                op0=ALU.max, op1=ALU.min)
                        else:
                            nc.scalar.activation(st[:, :], ps[:, :],
                                                 AF.Sigmoid)
                        sig.append(st)

                    u1 = sB.tile([128, 512], BF16, tag="u1", name="u1")
                    nc.vector.tensor_mul(u1[:, :], sig[1][:, :], c_t[:, :])
                    u2 = sB.tile([128, 512], BF16, tag="u2", name="u2")
                    nc.vector.tensor_mul(u2[:, :], sig[2][:, :], sig[0][:, :])
                    cc_t = sB.tile([128, 512], BF16, tag="cc", name="cc")
                    nc.vector.tensor_add(cc_t[:, :], u1[:, :], u2[:, :])
                    nc.sync.dma_start(
                        out=cc_ap[128 * m:128 * (m + 1),
                                  512 * n:512 * (n + 1)],
                        in_=cc_t[:])
                    rcc = sB.tile([128, 512], BF16, tag="rcc", name="rcc")
                    nc.vector.tensor_scalar(
                        out=rcc[:, :], in0=cc_t[:, :],
                        scalar1=0.0, scalar2=6.0, op0=ALU.max, op1=ALU.min)
                    ch_t = sB.tile([128, 512], BF16, tag="ch", name="ch")
                    nc.vector.tensor_mul(ch_t[:, :], rcc[:, :], sig[3][:, :])
                    nc.sync.dma_start(
                        out=ch_ap[128 * m:128 * (m + 1),
                                  512 * n:512 * (n + 1)],
                        in_=ch_t[:])

            # ---- software pipeline: B lags A by 2 slabs ----
            ins = {0: emit_slab_inputs(0)}
            wg_t, dwi_t = load_wB()
            ins[1] = emit_slab_inputs(1)

            iwq = {}
            for n in range(NCHUNK + 2):
                if n < NCHUNK:
                    if n not in ins:
                        ins[n] = emit_slab_inputs(n)
                    emit_A(n, *ins.pop(n))
                    if n + 1 < NCHUNK and n + 1 not in ins:
                        ins[n + 1] = emit_slab_inputs(n + 1)
                    # C(k) ready once A(k+1) wrote row 8k+9 (or A7 for k=7)
                    if n >= 1:
                        iwq[n - 1] = emit_C(n - 1)
                    if n == NCHUNK - 1:
                        iwq[n] = emit_C(n)
                if n >= 2:
                    emit_B(n - 2, iwq.pop(n - 2), wg_t, dwi_t)

    nc.compile()
    return nc


def pack_weights(W_dw, W_dwb, Wy, Wy_b, Wi, Wbi, Wbf, Wbc, Wbo):
    WyT = Wy[:, :, 0, 0].T.astype(np.float32)  # (832, 512) lhsT
    wy = np.zeros((128, 7, 512), np.float32)
    for k in range(4):  # h chunks first
        wy[:, k, :] = WyT[320 + 128 * k:320 + 128 * (k + 1), :]
    for k in range(2):
        wy[:, 4 + k, :] = WyT[128 * k:128 * (k + 1), :]
    wy[:64, 6, :] = WyT[256:320, :]

    wyb = (Wy_b + Wy[:, :320, 0, 0] @ W_dwb).astype(np.float32)
    wyb = wyb.reshape(4, 128).T.copy()

    def tile_pack(w):  # w: (128, 9) -> (128, 1152): [t*128 + J*32 + j]
        out = np.zeros((128, 1152), np.float32)
        for t in range(9):
            for Jb in range(4):
                j = np.arange(32)
                out[Jb * 0 + np.arange(4)[:, None] * 32 + j[None, :],
                    128 * t + 32 * Jb + j[None, :]] = w[32 * Jb + j, t]
        return out

    wdx = W_dw[:, 0].reshape(CIN, 9)
    dwx = np.stack([tile_pack(wdx[0:128]), tile_pack(wdx[128:256])])
    dwx = np.ascontiguousarray(dwx.transpose(1, 0, 2))  # (128, 2, 1152)

    # chunk2 (64 ch): block I holds diag for J = I % 2
    w2 = wdx[256:320]  # (64, 9)
    dwx2 = np.zeros((128, 288), np.float32)
    for t in range(9):
        for I in range(4):
            j = np.arange(32)
            dwx2[32 * I + j, 32 * t + j] = w2[32 * (I % 2) + j, t]

    wdi = Wi[:, 0].reshape(CH, 9)
    dwi = np.stack([tile_pack(wdi[128 * c:128 * (c + 1)]) for c in range(4)])
    dwi = np.ascontiguousarray(dwi.transpose(1, 0, 2))  # (128, 4, 1152)

    wg = np.zeros((128, 16, 512), np.float32)
    for g, W in enumerate([Wbi, Wbf, Wbc, Wbo]):
        lhsT = W[:, :, 0, 0].T.astype(np.float32)  # (512 in, 512 out)
        for k in range(4):
            wg[:, 4 * g + k, :] = lhsT[128 * k:128 * (k + 1), :]

    bf = lambda a: np.ascontiguousarray(a).astype(NPBF16)
    return {
        "wy": bf(wy), "wyb": np.ascontiguousarray(wyb), "wg": bf(wg),
        "dwx": bf(dwx), "dwx2": bf(dwx2), "dwi": bf(dwi),
    }


_CACHE = {}


def _get_nc():
    if "nc" not in _CACHE:
        _CACHE["nc"] = build_nc()
    return _CACHE["nc"]


def run(inputs, trace=False, tmpdir=None):
    """inputs: dict as from setup_inputs(). Returns ((ch, cc), results_obj)."""
    inp = {k: np.asarray(v, np.float32) for k, v in inputs.items()}
    packed = pack_weights(
        inp["W_dw"], inp["W_dwb"], inp["Wy"], inp["Wy_b"], inp["Wi"],
        inp["Wbi"], inp["Wbf"], inp["Wbc"], inp["Wbo"],
    )
    xpad_host = np.zeros((B, CIN, 66, 66), NPBF16)
    xpad_host[:, :, 1:65, 1:65] = inp["x"].astype(NPBF16)
    h_host = inp["h"].reshape(B, CH, PIX).astype(NPBF16)
    c_host = inp["c"].reshape(B, CH, PIX).astype(NPBF16)
    in_maps = []
    for b in range(B):
        in_maps.append({
            "x": xpad_host[b],
            "h": np.ascontiguousarray(h_host[b]),
            "c": np.ascontiguousarray(c_host[b]),
            **packed,
        })
    nc = _get_nc()
    kwargs = {}
    if trace:
        _enable_trace_hooks()
        kwargs = dict(trace=True, trace_cores=[0])
        if tmpdir:
            kwargs["tmpdir"] = tmpdir
    res = run_bass_kernel_spmd(nc, in_maps, core_ids=list(range(NCORES)), **kwargs)
    ch = np.stack([res.results[b]["och"].astype(np.float32).reshape(CH, HW, HW)
                   for b in range(B)])
    cc = np.stack([res.results[b]["occ"].astype(np.float32).reshape(CH, HW, HW)
                   for b in range(B)])
    return (ch, cc), res


def kernel(**inputs):
    (ch, cc), _ = run(inputs, trace=False)
    return ch, cc


# ---------- optional NTFF tracing support (test harness only) ----------

def _enable_trace_hooks():
    import types, ctypes, contextlib
    if "antenv.axon_hooks" in sys.modules:
        return
    import concourse.bass_utils as bass_utils

    def _ntff_profile_via_ctypes(so_path):
        lib = ctypes.CDLL(so_path)
        if not hasattr(lib, "axon_start_nrt_profile"):
            return None
        lib.axon_start_nrt_profile.argtypes = [
            ctypes.POINTER(ctypes.c_int64), ctypes.c_size_t]
        lib.axon_start_nrt_profile.restype = ctypes.c_int64
        lib.axon_stop_nrt_profile.argtypes = [ctypes.c_char_p]
        lib.axon_stop_nrt_profile.restype = ctypes.c_int64

        @contextlib.contextmanager
        def _hook(output_dir, device_ids):
            import jax
            jax.devices()
            if device_ids:
                ids = (ctypes.c_int64 * len(device_ids))(*device_ids)
                rc = lib.axon_start_nrt_profile(ids, len(device_ids))
            else:
                rc = lib.axon_start_nrt_profile(None, 0)
            if rc != 0:
                raise RuntimeError(f"axon_start_nrt_profile rc={rc}")
            try:
                yield
            finally:
                lib.axon_stop_nrt_profile(str(output_dir).encode())
        return _hook

    hook = _ntff_profile_via_ctypes("/opt/axon/libaxon_pjrt.so")
    mod = types.ModuleType("antenv.axon_hooks")
    mod.get_axon_ntff_profile_hook = lambda: hook
    mod.set_axon_ntff_profile_hook = lambda h: None
    sys.modules["antenv.axon_hooks"] = mod
    bass_utils.upload_artifacts = lambda tmpdir: "local://" + str(tmpdir)
